# revision 15
# baseline (speedup 1.0000x reference)
"""Trainium2 Bass kernel for nn_BatchPitNorm1d (pairwise Gaussian-CDF KDE +
inverse-normal transform).

Math:  u[b,f] = mean_s Phi((x[b,f] - c[s,f]) / bw[f]),  out = ndtri(u),
       bw = sigmoid(bw_param).

Algorithm: for fixed f, ndtri(u) is a smooth function H_f(x) of x alone, so
instead of B*S*F pairwise Phi evals the kernel:
  1. evaluates the raw erf-sums g_f(t) at N=8 Chebyshev nodes, sharded
     (4 node-groups) x (2 cdf-sample-halves) over 8 cores -- 2 nodes x 1024
     samples per core, one fused ACT erf instruction per node (fp16 samples,
     accum_out = free-dim sum, per-partition scale/bias from the bandwidth),
  2. AllGathers the raw sums (one small collective), adds the halves,
  3. applies ndtri at the nodes via a per-node degree-4 polynomial in
     ln(min-side mass) -- coefficients fitted offline over each node's
     provable mass window; one ACT Ln + 5 DVE ops total,
  4. converts H-at-nodes to per-feature degree-7 monomial coefficients with
     two fp32r PE matmuls that also broadcast them into scan order,
  5. evaluates the polynomial at local x with tensor_tensor_scan Horner
     segments (state = x*state + coeff, reset by a zero in data0).
Total error vs the f32 reference ~4.5e-3 rel L2 (tolerance 2e-2).

Layout: features (F=128) on partitions; x and cdf_data arrive pre-transposed
(feature-major) from the host shard step; cdf arrives fp16 with an 8-column
header carrying bw_param and the negated local nodes as bitcast f32 pairs.
"""

import math
from contextlib import ExitStack

import numpy as np

import concourse.bass as bass
import concourse.bacc as bacc
import concourse.tile as tile
from concourse import mybir
from concourse import bass_utils

F32 = mybir.dt.float32
F16 = mybir.dt.float16
F32R = mybir.dt.float32r

N_CORES = 8
B, S, F = 512, 2048, 128
BL = B // N_CORES            # 64 batch rows per core
N_CHEB = 8                   # Chebyshev nodes / polynomial order
NGRP = 4                     # node groups (core i: g = i % 4, h = i // 4)
NSPL = 2                     # sample splits
NLOC = N_CHEB // NGRP        # 2 nodes per core
SL = S // NSPL               # 1024 samples per core
DEG = 4                      # per-node ndtri poly degree (in ln m)
XDOM = 4.6                   # Chebyshev domain [-XDOM, XDOM] covers all x
HEAD = 8                     # fp16 header columns (p:2, -t:4, pad:2)
SEG = N_CHEB                 # scan segment length
SCF = SEG * BL               # 512: scan free size
SCH = SCF // 2               # 256: per-matmul / per-scan half

# ---- offline-fitted constants (see gen_consts.py) -------------------------
# fp16-rounded Chebyshev nodes, descending
NODES_T = np.array([4.5117188, 3.8242188, 2.5546875, 0.89746094, -0.89746094,
                    -2.5546875, -3.8242188, -4.5117188], dtype=np.float32)

# monomial fit matrix: alpha[f, j] = sum_n H[n, f] * CFIT[n, j]
CFIT = np.array([[-0.02480779, -0.025293207, 0.76806295, 0.7830917, -3.1724198, -3.234495, 3.0573344, 3.1171575],
 [0.08341817, 0.10034038, -2.548692, -3.0657198, 9.664615, 11.625178, -7.3861294, -8.88448],
 [-0.18711701, -0.33692506, 5.381087, 9.689247, -12.504128, -22.515078, 7.3936634, 13.313116],
 [0.62850666, 3.2214556, -3.600458, -18.454403, 6.0119333, 30.814594, -3.0648682, -15.7092],
 [0.62850666, -3.2214556, -3.600458, 18.454403, 6.0119333, -30.814594, -3.0648682, 15.7092],
 [-0.18711701, 0.33692506, 5.381087, -9.689247, -12.504128, 22.515078, 7.3936634, -13.313116],
 [0.08341817, -0.10034038, -2.548692, 3.0657198, 9.664615, -11.625178, -7.3861294, 8.88448],
 [-0.02480779, 0.025293207, 0.76806295, -0.7830917, -3.1724198, 3.234495, 3.0573344, -3.1171575]], dtype=np.float32)

# per-node ndtri-in-ln(m) coefficients, highest power first ([c4..c1, c0])
NDTRI_C = np.array([[-3.939184e-05, -0.0019820987, -0.04231281, -0.65028685, 0.053851865],
 [-0.0001070687, -0.004050899, -0.06547173, -0.7625096, -0.14434738],
 [-0.0008987558, -0.018857932, -0.1700141, -1.0925689, -0.5370468],
 [-0.033562798, -0.27378634, -0.93934083, -2.1693344, -1.1329428],
 [0.033562798, 0.27378634, 0.93934083, 2.1693344, 1.1329428],
 [0.0008987558, 0.018857932, 0.1700141, 1.0925689, 0.5370468],
 [0.0001070687, 0.004050899, 0.06547173, 0.7625096, 0.14434738],
 [3.939184e-05, 0.0019820987, 0.04231281, 0.65028685, -0.053851865]], dtype=np.float32)

# per-node clamp bounds for the raw erf-sum g (fit-window edges in g space)
G_LO = np.array([1998.2799, 1957.4087, 1719.2457, 175.83878, -1622.4263,
                 -2025.6102, -2047.8419, -2047.9973], dtype=np.float32)
G_HI = np.array([2047.9973, 2047.8419, 2025.6102, 1622.4263, -175.83878,
                 -1719.2457, -1957.4087, -1998.2799], dtype=np.float32)


def _consts_block():
    """[8, SCF + 16] f32: scan-ordered fit matrix + ndtri chain columns.

    cols 0..SCF-1: CB[n, b*SEG+k] = CFIT[n, SEG-1-k];
    cols SCF..SCF+4: ndtri chain coeffs c4..c1, c0; SCF+5: g_lo; SCF+6: g_hi;
    SCF+7: per-node Ln scale s_j/(2S); SCF+8: Ln bias 0.5; rest pad.
    """
    cb = np.zeros((N_CHEB, SCF + 16), dtype=np.float32)
    for k in range(SEG):
        cb[:, k:SCF:SEG] = CFIT[:, SEG - 1 - k][:, None]
    cb[:, SCF:SCF + DEG + 1] = NDTRI_C
    cb[:, SCF + DEG + 1] = G_LO
    cb[:, SCF + DEG + 2] = G_HI
    sj = np.where(NODES_T <= 0, 1.0, -1.0).astype(np.float32)
    cb[:, SCF + DEG + 3] = sj / np.float32(2.0 * S)
    cb[:, SCF + DEG + 4] = 0.5                        # Ln bias column
    return cb


def build(with_collective=True, stages=("load", "grid", "ndtri", "gather",
                                        "fit", "scan", "store")):
    stages = set(stages)
    ADD, MUL = mybir.AluOpType.add, mybir.AluOpType.mult
    MIN, MAX = mybir.AluOpType.min, mybir.AluOpType.max
    nc = bacc.Bacc("TRN2", target_bir_lowering=False, debug=False,
                   enable_asserts=False, num_devices=N_CORES)

    cdfh = nc.dram_tensor("cdfh", [F, HEAD + SL], F16, kind="ExternalInput")
    x_t = nc.dram_tensor("x_t", [F, BL], F32, kind="ExternalInput")
    out = nc.dram_tensor("out", [F, BL], F32, kind="ExternalOutput")
    cons_h = nc.inline_tensor(_consts_block(), name="consts")

    with tile.TileContext(nc) as tc, ExitStack() as ctx:
        io = ctx.enter_context(tc.tile_pool(name="io", bufs=2))
        small = ctx.enter_context(tc.tile_pool(name="small", bufs=1))
        nd = ctx.enter_context(tc.tile_pool(name="nd", bufs=2))
        psum = ctx.enter_context(tc.tile_pool(name="psum", bufs=2, space="PSUM"))
        dram = ctx.enter_context(tc.tile_pool(name="dram", bufs=1, space="DRAM"))

        # --- bulk loads.  cdfh first (longest transfer), consts second, x on
        # the Pool SWDGE queue.  ACT SEQ stays clear for table loads.
        cd_sb = io.tile([F, HEAD + SL], F16)
        if "load" in stages:
            nc.sync.dma_start(out=cd_sb, in_=cdfh[:, :])
        else:
            nc.vector.memset(cd_sb, 0.0)
        cons_sb = small.tile([N_CHEB, SCF + 16], F32)
        nc.sync.dma_start(out=cons_sb, in_=cons_h[:, :])
        x_sb = io.tile([F, BL], F32)
        nc.gpsimd.dma_start(out=x_sb, in_=x_t[:, :])
        # fp32r copy of the scan-ordered fit matrix for the PE matmuls
        # (on Pool so the DVE stays clear for the bias chain)
        cbr = small.tile([N_CHEB, SCF], F32)
        nc.gpsimd.tensor_copy(out=cbr.bitcast(F32R), in_=cons_sb[:, 0:SCF])

        # --- bandwidth scalars: neg_a = -1/(sigmoid(p)*sqrt(2)) via the
        # sigmoid table (same ACT table set as erf -> one table load)
        p_col = cd_sb[:, 0:2].bitcast(F32)               # [F, 1]
        negt = cd_sb[:, 2:2 + 2 * NLOC].bitcast(F32)     # [F, NLOC] = -t_j
        sig = small.tile([F, 1], F32)
        nc.scalar.activation(out=sig, in_=p_col,
                             func=mybir.ActivationFunctionType.Sigmoid)
        rcp = small.tile([F, 1], F32)
        nc.vector.reciprocal(out=rcp, in_=sig)
        neg_a = small.tile([F, 1], F32)
        nc.vector.tensor_scalar(out=neg_a, in0=rcp,
                                scalar1=-1.0 / math.sqrt(2.0), scalar2=None,
                                op0=MUL)
        bias_all = small.tile([F, NLOC], F32)            # a_f * t_j
        nc.vector.tensor_scalar_mul(out=bias_all, in0=negt, scalar1=neg_a)

        # --- epilogue prep on Pool (keeps DVE clear for the bias chain):
        # xt = clamp(x/XDOM); scan data0 x8[f, b*SEG+k] = 0 if k==0 else xt
        xt0 = small.tile([F, BL], F32)
        nc.gpsimd.tensor_scalar(out=xt0, in0=x_sb, scalar1=1.0 / XDOM,
                                scalar2=1.0, op0=MUL, op1=MIN)
        xt1 = small.tile([F, BL], F32)
        nc.gpsimd.tensor_scalar(out=xt1, in0=xt0, scalar1=-1.0, scalar2=None,
                                op0=MAX)
        x8 = small.tile([F, SCF], F32)
        x8_3d = x8.rearrange("f (b k) -> f b k", k=SEG)
        nc.gpsimd.memset(x8_3d[:, :, 0:1], 0.0)
        nc.gpsimd.tensor_copy(out=x8_3d[:, :, 1:SEG],
                              in_=xt1.unsqueeze(2).broadcast_to([F, BL, SEG - 1]))

        # --- grid pass: gacc[f, j] = sum_s erf(neg_a*c + a*t_j) (ACT)
        gacc = small.tile([F, NLOC], F32)
        scratch = psum.tile([128, SL], F32, tag="scr", bufs=1)
        if "grid" in stages:
            for j in range(NLOC):
                nc.scalar.activation(out=scratch, in_=cd_sb[:, HEAD:HEAD + SL],
                                     func=mybir.ActivationFunctionType.Erf,
                                     bias=bias_all[:, j:j + 1], scale=neg_a,
                                     accum_out=gacc[:, j:j + 1])
        else:
            nc.vector.memset(gacc, 0.0)

        # --- exchange: transpose-write local block (one DMA per node, fired
        # as its erf lands), AllGather, read all 16 (node, half) rows back in
        # one DMA as [8, 2F], add the halves
        cin = dram.tile([NLOC, F], F32)
        gat = nd.tile([N_CHEB, 2 * F], F32)
        if "gather" in stages:
            for j in range(NLOC):
                wj = bass.AP(tensor=cin.tensor, offset=cin.offset + j * F,
                             ap=[[1, F], [F, 1]])
                nc.sync.dma_start(out=wj, in_=gacc[:, j:j + 1])
            cout = dram.tile([N_CORES, NLOC, F], F32,
                             addr_space="Shared" if with_collective else "Local")
            if with_collective:
                nc.gpsimd.collective_compute(
                    "AllGather", mybir.AluOpType.bypass,
                    replica_groups=[list(range(N_CORES))],
                    ins=[cin.opt()], outs=[cout.opt()],
                )
                # core i holds (g = i % NGRP, h = i // NGRP): block i=g+4h at
                # offset i*NLOC*F, node n=2g+r row at n*F within the h-major
                # half.  gat[n, hF+f] <- cout[n*F + h*8F + f].
                src_ap = bass.AP(tensor=cout.tensor, offset=cout.offset,
                                 ap=[[F, N_CHEB], [N_CHEB * F, NSPL], [1, F]])
            else:  # stand-in: broadcast-read own block (timing model only)
                src_ap = bass.AP(tensor=cin.tensor, offset=cin.offset,
                                 ap=[[0, N_CHEB], [0, NSPL], [1, F]])
            nc.sync.dma_start(out=gat, in_=src_ap)
        else:
            nc.vector.memset(gat, 0.0)

        g = nd.tile([N_CHEB, F], F32)
        nc.vector.scalar_tensor_tensor(out=g, in0=gat[:, 0:F], scalar=0.0,
                                       in1=gat[:, F:2 * F],
                                       op0=ADD, op1=ADD)

        # warm the PE p-state while the ndtri chain runs (reads g so it fires
        # right after the gather lands, keeping pe_busy_start close)
        warm_ps = psum.tile([N_CHEB, N_CHEB], F32, tag="warm")
        nc.tensor.matmul(out=warm_ps, lhsT=g[:, 0:N_CHEB], rhs=g[:, 0:N_CHEB],
                         start=True, stop=True)

        # --- ndtri at nodes: clamp g, lnm = Ln(sj/(2S)*g + 0.5), then the
        # per-node chain (((c4*lnm + c3)*lnm + c2)*lnm + c1)*lnm + c0
        haug = nd.tile([N_CHEB, F], F32)
        ccol = lambda k: cons_sb[:, SCF + k:SCF + k + 1]  # noqa: E731
        if "ndtri" in stages:
            gcl = nd.tile([N_CHEB, F], F32)
            nc.vector.tensor_scalar(out=gcl, in0=g, scalar1=ccol(DEG + 2),
                                    scalar2=ccol(DEG + 1), op0=MIN, op1=MAX)
            lnm = nd.tile([N_CHEB, F], F32)
            nc.scalar.activation(out=lnm, in_=gcl,
                                 func=mybir.ActivationFunctionType.Ln,
                                 scale=ccol(DEG + 3), bias=ccol(DEG + 4))
            ch = nd.tile([N_CHEB, F], F32, name="ch0", tag="ch")
            nc.vector.tensor_scalar(out=ch, in0=lnm, scalar1=ccol(0),
                                    scalar2=None, op0=MUL)
            for k in range(1, DEG):
                dst = nd.tile([N_CHEB, F], F32, name=f"ch{k}", tag="ch")
                nc.vector.scalar_tensor_tensor(out=dst, in0=ch, scalar=ccol(k),
                                               in1=lnm, op0=ADD, op1=MUL)
                ch = dst
            nc.vector.tensor_scalar(out=haug.bitcast(F32R), in0=ch,
                                    scalar1=ccol(DEG), scalar2=None, op0=ADD)
        else:
            nc.vector.tensor_copy(out=haug.bitcast(F32R), in_=g)

        # --- fit + broadcast: alpha_bcast[f, t] = sum_n H[n,f]*CB[n,t],
        # two fp32r matmuls (one PSUM bank each) feeding the two scans
        alpha_ps = [psum.tile([128, SCH], F32, name=f"mm{h}", tag=f"mm{h}")
                    for h in range(2)]
        if "fit" in stages:
            for h in range(2):
                nc.tensor.matmul(out=alpha_ps[h], lhsT=haug.bitcast(F32R),
                                 rhs=cbr.bitcast(F32R)[:, h * SCH:(h + 1) * SCH],
                                 start=True, stop=True)
        else:
            for h in range(2):
                nc.vector.memset(alpha_ps[h], 0.0)

        # --- Horner scans (halves on DVE and Pool in parallel), one strided
        # gather of the segment tails, one store
        scano = small.tile([F, SCF], F32)
        if "scan" in stages:
            nc.vector.tensor_tensor_scan(
                out=scano[:, 0:SCH], data0=x8[:, 0:SCH],
                data1=alpha_ps[0], initial=0.0, op0=MUL, op1=ADD)
            nc.gpsimd.tensor_tensor_scan(
                out=scano[:, SCH:SCF], data0=x8[:, SCH:SCF],
                data1=alpha_ps[1], initial=0.0, op0=MUL, op1=ADD)
        else:
            nc.vector.memset(scano, 0.0)
        y = small.tile([F, BL], F32)
        nc.vector.tensor_copy(out=y, in_=scano[:, SEG - 1::SEG])
        if "store" in stages:
            nc.sync.dma_start(out=out[:, :], in_=y)

    nc.compile()
    return nc


_CACHE = {}


def _get_nc():
    if "nc" not in _CACHE:
        _CACHE["nc"] = build(with_collective=True)
    return _CACHE["nc"]


def kernel(x, cdf_data, bw_param):
    x = np.ascontiguousarray(x, dtype=np.float32)
    cdf_data = np.ascontiguousarray(cdf_data, dtype=np.float32)
    bw_param = np.ascontiguousarray(bw_param, dtype=np.float32)
    nc = _get_nc()

    xt = np.ascontiguousarray(x.T)                       # [F, B]
    cdf16 = cdf_data.astype(np.float16)
    cdf_halves = [np.ascontiguousarray(cdf16[h * SL:(h + 1) * SL].T)
                  for h in range(NSPL)]                   # each [F, SL] fp16
    p16 = bw_param[0].astype("<f4").view("<f2").reshape(F, 2)  # f32 bit pairs
    in_maps = []
    for i in range(N_CORES):
        g, h = i % NGRP, i // NGRP
        negt = (-NODES_T[g * NLOC:(g + 1) * NLOC]).astype("<f4").view("<f2")
        head = np.zeros((F, HEAD), dtype=np.float16)
        head[:, 0:2] = p16
        head[:, 2:2 + 2 * NLOC] = negt[None, :]
        cdfh = np.concatenate([head, cdf_halves[h]], axis=1)
        in_maps.append({
            "cdfh": np.ascontiguousarray(cdfh),
            "x_t": np.ascontiguousarray(xt[:, i * BL:(i + 1) * BL]),
        })
    res = bass_utils.run_bass_kernel_spmd(nc, in_maps,
                                          core_ids=list(range(N_CORES)))
    return np.concatenate([res.results[i]["out"].T for i in range(N_CORES)],
                          axis=0)


# revision 16
# speedup vs baseline: 1.0611x; 1.0611x over previous
"""Trainium2 Bass kernel for nn_BatchPitNorm1d (pairwise Gaussian-CDF KDE +
inverse-normal transform).

Math:  u[b,f] = mean_s Phi((x[b,f] - c[s,f]) / bw[f]),  out = ndtri(u),
       bw = sigmoid(bw_param).

Algorithm: for fixed f, ndtri(u) is a smooth function H_f(x) of x alone, so
instead of B*S*F pairwise Phi evals the kernel:
  1. evaluates the raw erf-sums g_f(t) at N=8 Chebyshev nodes, sharded
     (4 node-groups) x (2 cdf-sample-halves) over 8 cores -- 2 nodes x 1024
     samples per core, one fused ACT erf instruction per node (fp16 samples,
     accum_out = free-dim sum, per-partition scale/bias from the bandwidth),
  2. AllGathers the raw sums (one small collective), adds the halves,
  3. applies ndtri at the nodes via a per-node degree-4 polynomial in
     ln(min-side mass) -- coefficients fitted offline over each node's
     provable mass window; one ACT Ln + 5 DVE ops total,
  4. converts H-at-nodes to per-feature degree-7 monomial coefficients with
     two fp32r PE matmuls that also broadcast them into scan order,
  5. evaluates the polynomial at local x with tensor_tensor_scan Horner
     segments (state = x*state + coeff, reset by a zero in data0).
Total error vs the f32 reference ~4.5e-3 rel L2 (tolerance 2e-2).

Layout: features (F=128) on partitions; x and cdf_data arrive pre-transposed
(feature-major) from the host shard step; cdf arrives fp16 with an 8-column
header carrying bw_param and the negated local nodes as bitcast f32 pairs.
"""

import math
from contextlib import ExitStack

import numpy as np

import concourse.bass as bass
import concourse.bacc as bacc
import concourse.tile as tile
from concourse import mybir
from concourse import bass_utils

F32 = mybir.dt.float32
F16 = mybir.dt.float16
F32R = mybir.dt.float32r

N_CORES = 8
B, S, F = 512, 2048, 128
BL = B // N_CORES            # 64 batch rows per core
N_CHEB = 8                   # Chebyshev nodes / polynomial order
NGRP = 4                     # node groups (core i: g = i % 4, h = i // 4)
NSPL = 2                     # sample splits
NLOC = N_CHEB // NGRP        # 2 nodes per core
SL = S // NSPL               # 1024 samples per core
DEG = 4                      # per-node ndtri poly degree (in ln m)
XDOM = 4.6                   # Chebyshev domain [-XDOM, XDOM] covers all x
HEAD = 8                     # fp16 header columns (p:2, -t:4, pad:2)
SEG = N_CHEB                 # scan segment length
SCF = SEG * BL               # 512: scan free size
SCH = SCF // 2               # 256: per-matmul / per-scan half

# ---- offline-fitted constants (see gen_consts.py) -------------------------
# fp16-rounded Chebyshev nodes, descending
NODES_T = np.array([4.5117188, 3.8242188, 2.5546875, 0.89746094, -0.89746094,
                    -2.5546875, -3.8242188, -4.5117188], dtype=np.float32)

# monomial fit matrix: alpha[f, j] = sum_n H[n, f] * CFIT[n, j]
CFIT = np.array([[-0.02480779, -0.025293207, 0.76806295, 0.7830917, -3.1724198, -3.234495, 3.0573344, 3.1171575],
 [0.08341817, 0.10034038, -2.548692, -3.0657198, 9.664615, 11.625178, -7.3861294, -8.88448],
 [-0.18711701, -0.33692506, 5.381087, 9.689247, -12.504128, -22.515078, 7.3936634, 13.313116],
 [0.62850666, 3.2214556, -3.600458, -18.454403, 6.0119333, 30.814594, -3.0648682, -15.7092],
 [0.62850666, -3.2214556, -3.600458, 18.454403, 6.0119333, -30.814594, -3.0648682, 15.7092],
 [-0.18711701, 0.33692506, 5.381087, -9.689247, -12.504128, 22.515078, 7.3936634, -13.313116],
 [0.08341817, -0.10034038, -2.548692, 3.0657198, 9.664615, -11.625178, -7.3861294, 8.88448],
 [-0.02480779, 0.025293207, 0.76806295, -0.7830917, -3.1724198, 3.234495, 3.0573344, -3.1171575]], dtype=np.float32)

# per-node ndtri-in-ln(m) coefficients, highest power first ([c4..c1, c0])
NDTRI_C = np.array([[-3.939184e-05, -0.0019820987, -0.04231281, -0.65028685, 0.053851865],
 [-0.0001070687, -0.004050899, -0.06547173, -0.7625096, -0.14434738],
 [-0.0008987558, -0.018857932, -0.1700141, -1.0925689, -0.5370468],
 [-0.033562798, -0.27378634, -0.93934083, -2.1693344, -1.1329428],
 [0.033562798, 0.27378634, 0.93934083, 2.1693344, 1.1329428],
 [0.0008987558, 0.018857932, 0.1700141, 1.0925689, 0.5370468],
 [0.0001070687, 0.004050899, 0.06547173, 0.7625096, 0.14434738],
 [3.939184e-05, 0.0019820987, 0.04231281, 0.65028685, -0.053851865]], dtype=np.float32)

# per-node clamp bounds for the raw erf-sum g (fit-window edges in g space)
G_LO = np.array([1998.2799, 1957.4087, 1719.2457, 175.83878, -1622.4263,
                 -2025.6102, -2047.8419, -2047.9973], dtype=np.float32)
G_HI = np.array([2047.9973, 2047.8419, 2025.6102, 1622.4263, -175.83878,
                 -1719.2457, -1957.4087, -1998.2799], dtype=np.float32)


def _consts_block():
    """[8, SCF + 16] f32: scan-ordered fit matrix + ndtri chain columns.

    cols 0..SCF-1: CB[n, b*SEG+k] = CFIT[n, SEG-1-k];
    cols SCF..SCF+4: ndtri chain coeffs c4..c1, c0; SCF+5: g_lo; SCF+6: g_hi;
    SCF+7: per-node Ln scale s_j/(2S); SCF+8: Ln bias 0.5; rest pad.
    """
    cb = np.zeros((N_CHEB, SCF + 16), dtype=np.float32)
    for k in range(SEG):
        cb[:, k:SCF:SEG] = CFIT[:, SEG - 1 - k][:, None]
    cb[:, SCF:SCF + DEG + 1] = NDTRI_C
    cb[:, SCF + DEG + 1] = G_LO
    cb[:, SCF + DEG + 2] = G_HI
    sj = np.where(NODES_T <= 0, 1.0, -1.0).astype(np.float32)
    cb[:, SCF + DEG + 3] = sj / np.float32(2.0 * S)
    cb[:, SCF + DEG + 4] = 0.5                        # Ln bias column
    return cb


def build(with_collective=True, stages=("load", "grid", "ndtri", "gather",
                                        "fit", "scan", "store")):
    stages = set(stages)
    ADD, MUL = mybir.AluOpType.add, mybir.AluOpType.mult
    MIN, MAX = mybir.AluOpType.min, mybir.AluOpType.max
    nc = bacc.Bacc("TRN2", target_bir_lowering=False, debug=False,
                   enable_asserts=False, num_devices=N_CORES)

    cdfh = nc.dram_tensor("cdfh", [F, HEAD + SL], F16, kind="ExternalInput")
    x_t = nc.dram_tensor("x_t", [F, BL], F32, kind="ExternalInput")
    out = nc.dram_tensor("out", [F, BL], F32, kind="ExternalOutput")
    cons_h = nc.inline_tensor(_consts_block(), name="consts")

    with tile.TileContext(nc) as tc, ExitStack() as ctx:
        io = ctx.enter_context(tc.tile_pool(name="io", bufs=2))
        small = ctx.enter_context(tc.tile_pool(name="small", bufs=1))
        nd = ctx.enter_context(tc.tile_pool(name="nd", bufs=2))
        psum = ctx.enter_context(tc.tile_pool(name="psum", bufs=2, space="PSUM"))
        dram = ctx.enter_context(tc.tile_pool(name="dram", bufs=1, space="DRAM"))

        # --- bulk loads.  cdfh first (longest transfer), consts second, x on
        # the Pool SWDGE queue.  ACT SEQ stays clear for table loads.
        cd_sb = io.tile([F, HEAD + SL], F16)
        if "load" in stages:
            nc.sync.dma_start(out=cd_sb, in_=cdfh[:, :])
        else:
            nc.vector.memset(cd_sb, 0.0)
        cons_sb = small.tile([N_CHEB, SCF + 16], F32)
        nc.sync.dma_start(out=cons_sb, in_=cons_h[:, :])
        x_sb = io.tile([F, BL], F32)
        nc.gpsimd.dma_start(out=x_sb, in_=x_t[:, :])
        # fp32r copy of the scan-ordered fit matrix for the PE matmuls
        # (on Pool so the DVE stays clear for the bias chain)
        cbr = small.tile([N_CHEB, SCF], F32)
        nc.gpsimd.tensor_copy(out=cbr.bitcast(F32R), in_=cons_sb[:, 0:SCF])

        # --- bandwidth scalars: neg_a = -1/(sigmoid(p)*sqrt(2)) via the
        # sigmoid table (same ACT table set as erf -> one table load)
        p_col = cd_sb[:, 0:2].bitcast(F32)               # [F, 1]
        negt = cd_sb[:, 2:2 + 2 * NLOC].bitcast(F32)     # [F, NLOC] = -t_j
        sig = small.tile([F, 1], F32)
        nc.scalar.activation(out=sig, in_=p_col,
                             func=mybir.ActivationFunctionType.Sigmoid)
        rcp = small.tile([F, 1], F32)
        nc.vector.reciprocal(out=rcp, in_=sig)
        neg_a = small.tile([F, 1], F32)
        nc.vector.tensor_scalar(out=neg_a, in0=rcp,
                                scalar1=-1.0 / math.sqrt(2.0), scalar2=None,
                                op0=MUL)
        bias_all = small.tile([F, NLOC], F32)            # a_f * t_j
        nc.vector.tensor_scalar_mul(out=bias_all, in0=negt, scalar1=neg_a)

        # --- epilogue prep on Pool (keeps DVE clear for the bias chain):
        # xt = clamp(x/XDOM); scan data0 x8[f, b*SEG+k] = 0 if k==0 else xt
        xt0 = small.tile([F, BL], F32)
        nc.gpsimd.tensor_scalar(out=xt0, in0=x_sb, scalar1=1.0 / XDOM,
                                scalar2=1.0, op0=MUL, op1=MIN)
        xt1 = small.tile([F, BL], F32)
        nc.gpsimd.tensor_scalar(out=xt1, in0=xt0, scalar1=-1.0, scalar2=None,
                                op0=MAX)
        x8 = small.tile([F, SCF], F32)
        x8_3d = x8.rearrange("f (b k) -> f b k", k=SEG)
        nc.gpsimd.memset(x8_3d[:, :, 0:1], 0.0)
        nc.gpsimd.tensor_copy(out=x8_3d[:, :, 1:SEG],
                              in_=xt1.unsqueeze(2).broadcast_to([F, BL, SEG - 1]))

        # --- grid pass: gacc[f, j] = sum_s erf(neg_a*c + a*t_j) (ACT)
        gacc = small.tile([F, NLOC], F32)
        scratch = psum.tile([128, SL], F32, tag="scr", bufs=1)
        if "grid" in stages:
            for j in range(NLOC):
                nc.scalar.activation(out=scratch, in_=cd_sb[:, HEAD:HEAD + SL],
                                     func=mybir.ActivationFunctionType.Erf,
                                     bias=bias_all[:, j:j + 1], scale=neg_a,
                                     accum_out=gacc[:, j:j + 1])
        else:
            nc.vector.memset(gacc, 0.0)

        # --- exchange: transpose-write local block (one DMA per node, fired
        # as its erf lands), AllGather, read all 16 (node, half) rows back in
        # one DMA as [8, 2F], add the halves
        cin = dram.tile([NLOC, F], F32)
        gat = nd.tile([N_CHEB, 2 * F], F32)
        if "gather" in stages:
            for j in range(NLOC):
                wj = bass.AP(tensor=cin.tensor, offset=cin.offset + j * F,
                             ap=[[1, F], [F, 1]])
                nc.sync.dma_start(out=wj, in_=gacc[:, j:j + 1])
            cout = dram.tile([N_CORES, NLOC, F], F32,
                             addr_space="Shared" if with_collective else "Local")
            if with_collective:
                nc.gpsimd.collective_compute(
                    "AllGather", mybir.AluOpType.bypass,
                    replica_groups=[list(range(N_CORES))],
                    ins=[cin.opt()], outs=[cout.opt()],
                )
                # core i holds (g = i % NGRP, h = i // NGRP): block i=g+4h at
                # offset i*NLOC*F, node n=2g+r row at n*F within the h-major
                # half.  gat[n, hF+f] <- cout[n*F + h*8F + f].
                src_ap = bass.AP(tensor=cout.tensor, offset=cout.offset,
                                 ap=[[F, N_CHEB], [N_CHEB * F, NSPL], [1, F]])
            else:  # stand-in: broadcast-read own block (timing model only)
                src_ap = bass.AP(tensor=cin.tensor, offset=cin.offset,
                                 ap=[[0, N_CHEB], [0, NSPL], [1, F]])
            nc.sync.dma_start(out=gat, in_=src_ap)
        else:
            nc.vector.memset(gat, 0.0)

        g = nd.tile([N_CHEB, F], F32)
        nc.vector.scalar_tensor_tensor(out=g, in0=gat[:, 0:F], scalar=0.0,
                                       in1=gat[:, F:2 * F],
                                       op0=ADD, op1=ADD)

        # warm the PE p-state while the ndtri chain runs (reads g so it fires
        # right after the gather lands, keeping pe_busy_start close)
        warm_ps = psum.tile([N_CHEB, N_CHEB], F32, tag="warm")
        nc.tensor.matmul(out=warm_ps, lhsT=g[:, 0:N_CHEB], rhs=g[:, 0:N_CHEB],
                         start=True, stop=True)

        # --- ndtri at nodes: clamp g, lnm = Ln(sj/(2S)*g + 0.5), then the
        # per-node chain (((c4*lnm + c3)*lnm + c2)*lnm + c1)*lnm + c0
        haug = nd.tile([N_CHEB, F], F32)
        ccol = lambda k: cons_sb[:, SCF + k:SCF + k + 1]  # noqa: E731
        if "ndtri" in stages:
            gcl = nd.tile([N_CHEB, F], F32)
            nc.vector.tensor_scalar(out=gcl, in0=g, scalar1=ccol(DEG + 2),
                                    scalar2=ccol(DEG + 1), op0=MIN, op1=MAX)
            lnm = nd.tile([N_CHEB, F], F32)
            nc.scalar.activation(out=lnm, in_=gcl,
                                 func=mybir.ActivationFunctionType.Ln,
                                 scale=ccol(DEG + 3), bias=ccol(DEG + 4))
            ch = nd.tile([N_CHEB, F], F32, name="ch0", tag="ch")
            nc.vector.tensor_scalar(out=ch, in0=lnm, scalar1=ccol(0),
                                    scalar2=None, op0=MUL)
            for k in range(1, DEG):
                dst = nd.tile([N_CHEB, F], F32, name=f"ch{k}", tag="ch")
                nc.vector.scalar_tensor_tensor(out=dst, in0=ch, scalar=ccol(k),
                                               in1=lnm, op0=ADD, op1=MUL)
                ch = dst
            nc.vector.tensor_scalar(out=haug.bitcast(F32R), in0=ch,
                                    scalar1=ccol(DEG), scalar2=None, op0=ADD)
        else:
            nc.vector.tensor_copy(out=haug.bitcast(F32R), in_=g)

        # --- fit + broadcast: alpha_bcast[f, t] = sum_n H[n,f]*CB[n,t],
        # two fp32r matmuls (one PSUM bank each) feeding the two scans
        alpha_ps = [psum.tile([128, SCH], F32, name=f"mm{h}", tag=f"mm{h}")
                    for h in range(2)]
        if "fit" in stages:
            for h in range(2):
                nc.tensor.matmul(out=alpha_ps[h], lhsT=haug.bitcast(F32R),
                                 rhs=cbr.bitcast(F32R)[:, h * SCH:(h + 1) * SCH],
                                 start=True, stop=True)
        else:
            for h in range(2):
                nc.vector.memset(alpha_ps[h], 0.0)

        # --- Horner scans (Pool cannot read PSUM, so both on DVE; the second
        # pipelines behind the first), one strided gather, one store
        scano = small.tile([F, SCF], F32)
        if "scan" in stages:
            for h in range(2):
                nc.vector.tensor_tensor_scan(
                    out=scano[:, h * SCH:(h + 1) * SCH],
                    data0=x8[:, h * SCH:(h + 1) * SCH],
                    data1=alpha_ps[h], initial=0.0, op0=MUL, op1=ADD)
        else:
            nc.vector.memset(scano, 0.0)
        y = small.tile([F, BL], F32)
        nc.vector.tensor_copy(out=y, in_=scano[:, SEG - 1::SEG])
        if "store" in stages:
            nc.sync.dma_start(out=out[:, :], in_=y)

    nc.compile()
    return nc


_CACHE = {}


def _get_nc():
    if "nc" not in _CACHE:
        _CACHE["nc"] = build(with_collective=True)
    return _CACHE["nc"]


def kernel(x, cdf_data, bw_param):
    x = np.ascontiguousarray(x, dtype=np.float32)
    cdf_data = np.ascontiguousarray(cdf_data, dtype=np.float32)
    bw_param = np.ascontiguousarray(bw_param, dtype=np.float32)
    nc = _get_nc()

    xt = np.ascontiguousarray(x.T)                       # [F, B]
    cdf16 = cdf_data.astype(np.float16)
    cdf_halves = [np.ascontiguousarray(cdf16[h * SL:(h + 1) * SL].T)
                  for h in range(NSPL)]                   # each [F, SL] fp16
    p16 = bw_param[0].astype("<f4").view("<f2").reshape(F, 2)  # f32 bit pairs
    in_maps = []
    for i in range(N_CORES):
        g, h = i % NGRP, i // NGRP
        negt = (-NODES_T[g * NLOC:(g + 1) * NLOC]).astype("<f4").view("<f2")
        head = np.zeros((F, HEAD), dtype=np.float16)
        head[:, 0:2] = p16
        head[:, 2:2 + 2 * NLOC] = negt[None, :]
        cdfh = np.concatenate([head, cdf_halves[h]], axis=1)
        in_maps.append({
            "cdfh": np.ascontiguousarray(cdfh),
            "x_t": np.ascontiguousarray(xt[:, i * BL:(i + 1) * BL]),
        })
    res = bass_utils.run_bass_kernel_spmd(nc, in_maps,
                                          core_ids=list(range(N_CORES)))
    return np.concatenate([res.results[i]["out"].T for i in range(N_CORES)],
                          axis=0)


# revision 19
# speedup vs baseline: 1.0758x; 1.0138x over previous
"""Trainium2 Bass kernel for nn_BatchPitNorm1d (pairwise Gaussian-CDF KDE +
inverse-normal transform).

Math:  u[b,f] = mean_s Phi((x[b,f] - c[s,f]) / bw[f]),  out = ndtri(u),
       bw = sigmoid(bw_param).

Algorithm: for fixed f, ndtri(u) is a smooth function H_f(x) of x alone, so
instead of B*S*F pairwise Phi evals the kernel:
  1. evaluates the raw erf-sums g_f(t) at N=8 Chebyshev nodes, sharded
     (4 node-groups) x (2 cdf-sample-halves) over 8 cores -- 2 nodes x 1024
     samples per core, one fused ACT erf instruction per node (fp16 samples,
     accum_out = free-dim sum, per-partition scale/bias from the bandwidth),
  2. AllGathers the raw sums (one small collective), adds the halves,
  3. applies ndtri at the nodes via a per-node degree-4 polynomial in
     ln(min-side mass) -- coefficients fitted offline over each node's
     provable mass window; one ACT Ln + 5 DVE ops total,
  4. converts H-at-nodes to per-feature degree-7 monomial coefficients with
     two fp32r PE matmuls that also broadcast them into scan order,
  5. evaluates the polynomial at local x with tensor_tensor_scan Horner
     segments (state = x*state + coeff, reset by a zero in data0).
Total error vs the f32 reference ~4.5e-3 rel L2 (tolerance 2e-2).

Layout: features (F=128) on partitions; x and cdf_data arrive pre-transposed
(feature-major) from the host shard step; cdf arrives fp16 with an 8-column
header carrying bw_param and the negated local nodes as bitcast f32 pairs.
"""

import math
from contextlib import ExitStack

import numpy as np

import concourse.bass as bass
import concourse.bacc as bacc
import concourse.tile as tile
from concourse import mybir
from concourse import bass_utils

F32 = mybir.dt.float32
F16 = mybir.dt.float16
F32R = mybir.dt.float32r

N_CORES = 8
B, S, F = 512, 2048, 128
BL = B // N_CORES            # 64 batch rows per core
N_CHEB = 8                   # Chebyshev nodes / polynomial order
NGRP = 4                     # node groups (core i: g = i % 4, h = i // 4)
NSPL = 2                     # sample splits
NLOC = N_CHEB // NGRP        # 2 nodes per core
SL = S // NSPL               # 1024 samples per core
DEG = 4                      # per-node ndtri poly degree (in ln m)
XDOM = 4.6                   # Chebyshev domain [-XDOM, XDOM] covers all x
HEAD = 8                     # fp16 header columns (p:2, -t:4, pad:2)
SEG = N_CHEB                 # scan segment length
SCF = SEG * BL               # 512: scan free size
SCH = SCF // 2               # 256: per-matmul / per-scan half

# ---- offline-fitted constants (see gen_consts.py) -------------------------
# fp16-rounded Chebyshev nodes, descending
NODES_T = np.array([4.5117188, 3.8242188, 2.5546875, 0.89746094, -0.89746094,
                    -2.5546875, -3.8242188, -4.5117188], dtype=np.float32)

# monomial fit matrix: alpha[f, j] = sum_n H[n, f] * CFIT[n, j]
CFIT = np.array([[-0.02480779, -0.025293207, 0.76806295, 0.7830917, -3.1724198, -3.234495, 3.0573344, 3.1171575],
 [0.08341817, 0.10034038, -2.548692, -3.0657198, 9.664615, 11.625178, -7.3861294, -8.88448],
 [-0.18711701, -0.33692506, 5.381087, 9.689247, -12.504128, -22.515078, 7.3936634, 13.313116],
 [0.62850666, 3.2214556, -3.600458, -18.454403, 6.0119333, 30.814594, -3.0648682, -15.7092],
 [0.62850666, -3.2214556, -3.600458, 18.454403, 6.0119333, -30.814594, -3.0648682, 15.7092],
 [-0.18711701, 0.33692506, 5.381087, -9.689247, -12.504128, 22.515078, 7.3936634, -13.313116],
 [0.08341817, -0.10034038, -2.548692, 3.0657198, 9.664615, -11.625178, -7.3861294, 8.88448],
 [-0.02480779, 0.025293207, 0.76806295, -0.7830917, -3.1724198, 3.234495, 3.0573344, -3.1171575]], dtype=np.float32)

# per-node ndtri-in-ln(m) coefficients, highest power first ([c4..c1, c0])
NDTRI_C = np.array([[-3.939184e-05, -0.0019820987, -0.04231281, -0.65028685, 0.053851865],
 [-0.0001070687, -0.004050899, -0.06547173, -0.7625096, -0.14434738],
 [-0.0008987558, -0.018857932, -0.1700141, -1.0925689, -0.5370468],
 [-0.033562798, -0.27378634, -0.93934083, -2.1693344, -1.1329428],
 [0.033562798, 0.27378634, 0.93934083, 2.1693344, 1.1329428],
 [0.0008987558, 0.018857932, 0.1700141, 1.0925689, 0.5370468],
 [0.0001070687, 0.004050899, 0.06547173, 0.7625096, 0.14434738],
 [3.939184e-05, 0.0019820987, 0.04231281, 0.65028685, -0.053851865]], dtype=np.float32)

# per-node clamp bounds for the raw erf-sum g (fit-window edges in g space)
G_LO = np.array([1998.2799, 1957.4087, 1719.2457, 175.83878, -1622.4263,
                 -2025.6102, -2047.8419, -2047.9973], dtype=np.float32)
G_HI = np.array([2047.9973, 2047.8419, 2025.6102, 1622.4263, -175.83878,
                 -1719.2457, -1957.4087, -1998.2799], dtype=np.float32)


def _consts_block():
    """[8, SCF + 16] f32: scan-ordered fit matrix + ndtri chain columns.

    cols 0..SCF-1: CB[n, b*SEG+k] = CFIT[n, SEG-1-k];
    cols SCF..SCF+4: ndtri chain coeffs c4..c1, c0; SCF+5: g_lo; SCF+6: g_hi;
    SCF+7: per-node Ln scale s_j/(2S); SCF+8: Ln bias 0.5; rest pad.
    """
    cb = np.zeros((N_CHEB, SCF + 16), dtype=np.float32)
    for k in range(SEG):
        cb[:, k:SCF:SEG] = CFIT[:, SEG - 1 - k][:, None]
    cb[:, SCF:SCF + DEG + 1] = NDTRI_C
    cb[:, SCF + DEG + 1] = G_LO
    cb[:, SCF + DEG + 2] = G_HI
    sj = np.where(NODES_T <= 0, 1.0, -1.0).astype(np.float32)
    cb[:, SCF + DEG + 3] = sj / np.float32(2.0 * S)
    cb[:, SCF + DEG + 4] = 0.5                        # Ln bias column
    return cb


def build(with_collective=True, stages=("load", "grid", "ndtri", "gather",
                                        "fit", "scan", "store")):
    stages = set(stages)
    ADD, MUL = mybir.AluOpType.add, mybir.AluOpType.mult
    MIN, MAX = mybir.AluOpType.min, mybir.AluOpType.max
    nc = bacc.Bacc("TRN2", target_bir_lowering=False, debug=False,
                   enable_asserts=False, num_devices=N_CORES)

    cdfh = nc.dram_tensor("cdfh", [F, HEAD + SL], F16, kind="ExternalInput")
    x_t = nc.dram_tensor("x_t", [F, BL], F32, kind="ExternalInput")
    out = nc.dram_tensor("out", [F, BL], F32, kind="ExternalOutput")
    cons_h = nc.inline_tensor(_consts_block(), name="consts")

    with tile.TileContext(nc) as tc, ExitStack() as ctx:
        io = ctx.enter_context(tc.tile_pool(name="io", bufs=2))
        small = ctx.enter_context(tc.tile_pool(name="small", bufs=1))
        nd = ctx.enter_context(tc.tile_pool(name="nd", bufs=2))
        psum = ctx.enter_context(tc.tile_pool(name="psum", bufs=2, space="PSUM"))
        dram = ctx.enter_context(tc.tile_pool(name="dram", bufs=1, space="DRAM"))

        # --- bulk loads.  cdfh first (longest transfer), consts second, x on
        # the Pool SWDGE queue.  ACT SEQ stays clear for table loads.
        cd_sb = io.tile([F, HEAD + SL], F16)
        if "load" in stages:
            nc.sync.dma_start(out=cd_sb, in_=cdfh[:, :])
        else:
            nc.vector.memset(cd_sb, 0.0)
        cons_sb = small.tile([N_CHEB, SCF + 16], F32)
        nc.sync.dma_start(out=cons_sb, in_=cons_h[:, :])
        x_sb = io.tile([F, BL], F32)
        nc.gpsimd.dma_start(out=x_sb, in_=x_t[:, :])
        # fp32r copy of the scan-ordered fit matrix for the PE matmuls
        # (on Pool so the DVE stays clear for the bias chain)
        cbr = small.tile([N_CHEB, SCF], F32)
        nc.gpsimd.tensor_copy(out=cbr.bitcast(F32R), in_=cons_sb[:, 0:SCF])

        # --- bandwidth scalars: neg_a = -1/(sigmoid(p)*sqrt(2)) via the
        # sigmoid table (same ACT table set as erf -> one table load)
        p_col = cd_sb[:, 0:2].bitcast(F32)               # [F, 1]
        negt = cd_sb[:, 2:2 + 2 * NLOC].bitcast(F32)     # [F, NLOC] = -t_j
        sig = small.tile([F, 1], F32)
        nc.scalar.activation(out=sig, in_=p_col,
                             func=mybir.ActivationFunctionType.Sigmoid)
        rcp = small.tile([F, 1], F32)
        nc.vector.reciprocal(out=rcp, in_=sig)
        neg_a = small.tile([F, 1], F32)
        nc.vector.tensor_scalar(out=neg_a, in0=rcp,
                                scalar1=-1.0 / math.sqrt(2.0), scalar2=None,
                                op0=MUL)
        bias_all = small.tile([F, NLOC], F32)            # a_f * t_j
        nc.vector.tensor_scalar_mul(out=bias_all, in0=negt, scalar1=neg_a)

        # --- epilogue prep on Pool (keeps DVE clear for the bias chain):
        # xt = clamp(x/XDOM); scan data0 x8[f, b*SEG+k] = 0 if k==0 else xt
        xt0 = small.tile([F, BL], F32)
        nc.gpsimd.tensor_scalar(out=xt0, in0=x_sb, scalar1=1.0 / XDOM,
                                scalar2=1.0, op0=MUL, op1=MIN)
        xt1 = small.tile([F, BL], F32)
        nc.gpsimd.tensor_scalar(out=xt1, in0=xt0, scalar1=-1.0, scalar2=None,
                                op0=MAX)
        x8 = small.tile([F, SCF], F32)
        x8_3d = x8.rearrange("f (b k) -> f b k", k=SEG)
        nc.gpsimd.memset(x8_3d[:, :, 0:1], 0.0)
        nc.gpsimd.tensor_copy(out=x8_3d[:, :, 1:SEG],
                              in_=xt1.unsqueeze(2).broadcast_to([F, BL, SEG - 1]))

        # --- grid pass: gacc[f, j] = sum_s erf(neg_a*c + a*t_j) (ACT)
        gacc = small.tile([F, NLOC], F32)
        scratch = psum.tile([128, SL], F32, tag="scr", bufs=1)
        if "grid" in stages:
            for j in range(NLOC):
                nc.scalar.activation(out=scratch, in_=cd_sb[:, HEAD:HEAD + SL],
                                     func=mybir.ActivationFunctionType.Erf,
                                     bias=bias_all[:, j:j + 1], scale=neg_a,
                                     accum_out=gacc[:, j:j + 1])
        else:
            nc.vector.memset(gacc, 0.0)

        # --- exchange: transpose-write local block (one DMA per node, fired
        # as its erf lands), AllGather, read all 16 (node, half) rows back in
        # one DMA as [8, 2F], add the halves
        cin = dram.tile([NLOC, F], F32)
        gat = nd.tile([N_CHEB, 2 * F], F32)
        if "gather" in stages:
            for j in range(NLOC):
                wj = bass.AP(tensor=cin.tensor, offset=cin.offset + j * F,
                             ap=[[1, F], [F, 1]])
                nc.sync.dma_start(out=wj, in_=gacc[:, j:j + 1])
            cout = dram.tile([N_CORES, NLOC, F], F32,
                             addr_space="Shared" if with_collective else "Local")
            if with_collective:
                nc.gpsimd.collective_compute(
                    "AllGather", mybir.AluOpType.bypass,
                    replica_groups=[list(range(N_CORES))],
                    ins=[cin.opt()], outs=[cout.opt()],
                )
                # core i holds (g = i % NGRP, h = i // NGRP): block i=g+4h at
                # offset i*NLOC*F, node n=2g+r row at n*F within the h-major
                # half.  gat[n, hF+f] <- cout[n*F + h*8F + f].
                src_ap = bass.AP(tensor=cout.tensor, offset=cout.offset,
                                 ap=[[F, N_CHEB], [N_CHEB * F, NSPL], [1, F]])
            else:  # stand-in: broadcast-read own block (timing model only)
                src_ap = bass.AP(tensor=cin.tensor, offset=cin.offset,
                                 ap=[[0, N_CHEB], [0, NSPL], [1, F]])
            nc.sync.dma_start(out=gat, in_=src_ap)
        else:
            nc.vector.memset(gat, 0.0)

        g = nd.tile([N_CHEB, F], F32)
        nc.vector.scalar_tensor_tensor(out=g, in0=gat[:, 0:F], scalar=0.0,
                                       in1=gat[:, F:2 * F],
                                       op0=ADD, op1=ADD)

        # warm the PE p-state while the ndtri chain runs (reads g so it fires
        # right after the gather lands, keeping pe_busy_start close)
        warm_ps = psum.tile([N_CHEB, N_CHEB], F32, tag="warm")
        nc.tensor.matmul(out=warm_ps, lhsT=g[:, 0:N_CHEB], rhs=g[:, 0:N_CHEB],
                         start=True, stop=True)

        # --- ndtri at nodes: lnm = Ln(sj/(2S)*g + 0.5), then the per-node
        # chain (((c4*lnm + c3)*lnm + c2)*lnm + c1)*lnm + c0.  No clamp: the
        # fit windows are 1.6-32x wider than the provable mass range, and
        # m > 0 holds structurally (|g| < 2S), so Ln stays finite.
        haug = nd.tile([N_CHEB, F], F32)
        ccol = lambda k: cons_sb[:, SCF + k:SCF + k + 1]  # noqa: E731
        if "ndtri" in stages:
            lnm = nd.tile([N_CHEB, F], F32)
            nc.scalar.activation(out=lnm, in_=g,
                                 func=mybir.ActivationFunctionType.Ln,
                                 scale=ccol(DEG + 3), bias=ccol(DEG + 4))
            ch = nd.tile([N_CHEB, F], F32, name="ch0", tag="ch")
            nc.vector.tensor_scalar(out=ch, in0=lnm, scalar1=ccol(0),
                                    scalar2=None, op0=MUL)
            for k in range(1, DEG):
                dst = nd.tile([N_CHEB, F], F32, name=f"ch{k}", tag="ch")
                nc.vector.scalar_tensor_tensor(out=dst, in0=ch, scalar=ccol(k),
                                               in1=lnm, op0=ADD, op1=MUL)
                ch = dst
            nc.vector.tensor_scalar(out=haug.bitcast(F32R), in0=ch,
                                    scalar1=ccol(DEG), scalar2=None, op0=ADD)
        else:
            nc.vector.tensor_copy(out=haug.bitcast(F32R), in_=g)

        # --- fit + broadcast: alpha_bcast[f, t] = sum_n H[n,f]*CB[n,t],
        # two fp32r matmuls (one PSUM bank each) feeding the two scans
        alpha_ps = [psum.tile([128, SCH], F32, name=f"mm{h}", tag=f"mm{h}")
                    for h in range(2)]
        if "fit" in stages:
            for h in range(2):
                nc.tensor.matmul(out=alpha_ps[h], lhsT=haug.bitcast(F32R),
                                 rhs=cbr.bitcast(F32R)[:, h * SCH:(h + 1) * SCH],
                                 start=True, stop=True)
        else:
            for h in range(2):
                nc.vector.memset(alpha_ps[h], 0.0)

        # --- Horner scans (Pool cannot read PSUM, so both on DVE; the second
        # pipelines behind the first), one strided gather, one store
        scano = small.tile([F, SCF], F32)
        if "scan" in stages:
            for h in range(2):
                nc.vector.tensor_tensor_scan(
                    out=scano[:, h * SCH:(h + 1) * SCH],
                    data0=x8[:, h * SCH:(h + 1) * SCH],
                    data1=alpha_ps[h], initial=0.0, op0=MUL, op1=ADD)
        else:
            nc.vector.memset(scano, 0.0)
        y = small.tile([F, BL], F32)
        nc.vector.tensor_copy(out=y, in_=scano[:, SEG - 1::SEG])
        if "store" in stages:
            nc.sync.dma_start(out=out[:, :], in_=y)

    nc.compile()
    return nc


_CACHE = {}


def _get_nc():
    if "nc" not in _CACHE:
        _CACHE["nc"] = build(with_collective=True)
    return _CACHE["nc"]


def kernel(x, cdf_data, bw_param):
    x = np.ascontiguousarray(x, dtype=np.float32)
    cdf_data = np.ascontiguousarray(cdf_data, dtype=np.float32)
    bw_param = np.ascontiguousarray(bw_param, dtype=np.float32)
    nc = _get_nc()

    xt = np.ascontiguousarray(x.T)                       # [F, B]
    cdf16 = cdf_data.astype(np.float16)
    cdf_halves = [np.ascontiguousarray(cdf16[h * SL:(h + 1) * SL].T)
                  for h in range(NSPL)]                   # each [F, SL] fp16
    p16 = bw_param[0].astype("<f4").view("<f2").reshape(F, 2)  # f32 bit pairs
    in_maps = []
    for i in range(N_CORES):
        g, h = i % NGRP, i // NGRP
        negt = (-NODES_T[g * NLOC:(g + 1) * NLOC]).astype("<f4").view("<f2")
        head = np.zeros((F, HEAD), dtype=np.float16)
        head[:, 0:2] = p16
        head[:, 2:2 + 2 * NLOC] = negt[None, :]
        cdfh = np.concatenate([head, cdf_halves[h]], axis=1)
        in_maps.append({
            "cdfh": np.ascontiguousarray(cdfh),
            "x_t": np.ascontiguousarray(xt[:, i * BL:(i + 1) * BL]),
        })
    res = bass_utils.run_bass_kernel_spmd(nc, in_maps,
                                          core_ids=list(range(N_CORES)))
    return np.concatenate([res.results[i]["out"].T for i in range(N_CORES)],
                          axis=0)
